# revision 9
# baseline (speedup 1.0000x reference)
"""Trainium2 Bass kernel for windowless relative-position-bias attention.

Problem (hardcoded shapes):
  x [16, 1024, 512] f32, W_qkv [512, 1536], rel_table [3969, 8],
  W_out [512, 512], b_out [512], rel_index [1048576] i32 (canonical
  32x32 relative-position pattern; only its structure is used).

Sharding: pure data-parallel over batch -- core c owns batches 2c and
2c+1 end-to-end (all 8 heads + output projection).  No collectives at
all, so no cross-core skew sensitivity and no AllToAll stalls.

Per core the 16 (head, batch) steps run as a software pipeline keyed
on the PE instruction stream: step i emits its dots+PV, step i+1's
q/k/v projections as PE filler, and step i-1's normalization; step
i+1's first two key-blocks ride the tail of step i so the PE never
idles across step boundaries (keeps the HAM clock gate at 2.4 GHz).
Dots are emitted as [128, 512] halves so softmax latency decouples
from PE pacing with only 2 PSUM banks of dots in flight.

Softmax (max-subtraction skipped; logits bounded ~|9|) splits the 8
key-blocks per step across three engines:
  * mc 2/3/6/7: ScalarE exp(SCALE*dots), then multiply with the
    host-precomputed exp(bias) on VectorE (mc 2) or GpSimd (3, 6, 7).
  * mc 0/1/4/5: fused Schraudolph exp on VectorE -- one custom DVE op
    i16 = (dots*SCALE*A + C1) + A*bias, bits reinterpreted as fp16
    (A = 1024*log2(e); C1 centers the piecewise-linear exp2 error so
    softmax ratios stay unbiased).
The PV matmul gets the softmax denominator for free via a ones-column
appended to v; normalization is a K=1 bf16 broadcast matmul +
reciprocal + multiply, deferred one step so its latency hides.

The per-head [1024, 1024] bias tables (exp(bias) on A-path key-block
rows, A*bias on the Schraudolph rows) are host-expanded, streamed one
2 MB dma_start per head on the scalar HWDGE ring (separate from the
sync ring so table streaming never delays the small latency-critical
copies), double-buffered two steps ahead.
"""

import os
import sys

for _p in ("/opt/trn_rl_repo", "/root/.axon_site/_ro/trn_rl_repo"):
    if os.path.isdir(_p) and _p not in sys.path:
        sys.path.insert(0, _p)

import numpy as np
import ml_dtypes

import concourse.bass as bass
import concourse.mybir as mybir
import concourse.tile as tile
from concourse import bacc
from concourse.bass import AP
from concourse.bass_utils import run_bass_kernel_spmd

# Content-hash NEFF cache: identical BIR -> reuse the compiled NEFF
# (neuronxcc is ~6 min; this makes repeat runs seconds).
import concourse.bass_utils as _bu
import concourse.bass2jax as _b2j

_orig_compile_bir = _bu.compile_bir_kernel


def _cached_compile_bir(bir_json, tmpdir, neff_name="file.neff"):
    import hashlib
    import shutil
    h = hashlib.sha256(bir_json).hexdigest()[:24]
    cdir = os.environ.get("NEFF_CACHE_DIR", "/tmp/neff_cache")
    os.makedirs(cdir, exist_ok=True)
    cpath = os.path.join(cdir, h + ".neff")
    if os.path.exists(cpath):
        dst = os.path.join(tmpdir, neff_name)
        shutil.copy(cpath, dst)
        return dst
    p = _orig_compile_bir(bir_json, tmpdir, neff_name)
    try:
        shutil.copy(p, cpath)
    except OSError:
        pass
    return p


_bu.compile_bir_kernel = _cached_compile_bir
_b2j.compile_bir_kernel = _cached_compile_bir

B, IH, IW = 16, 32, 32
N = IH * IW          # 1024
H, D = 8, 64
INNER = H * D        # 512
INP = OUP = 512
SCALE = D ** -0.5    # 0.125
NCORES = 8
BPC = B // NCORES    # batches per core = 2
TBL = (2 * IH - 1) * (2 * IW - 1)  # 3969

# Schraudolph fp16 exp constants
A_EXP = 1024.0 * np.log2(np.e)          # 1477.3197
C1_EXP = 15360.0 - 59.0 + 0.5           # exponent offset - mean-center - trunc

# per-step key-block engine assignment (independent of the row-tile
# parity): mc 0/1/4/5 take the one-op DVE Schraudolph path (fast --
# mc 0/1 feed the earliest PV matmuls); mc 2/3/6/7 take the two-op
# ScalarE-exp + multiply path (mult on VectorE for mc2, GpSimd else).
A_MC = (2, 3, 6, 7)
D_MC = (0, 1, 4, 5)

F32 = mybir.dt.float32
F16 = mybir.dt.float16
BF16 = mybir.dt.bfloat16
I16 = mybir.dt.int16

STEPS = [(h, b) for h in range(H) for b in range(BPC)]  # h-major, b-inner
NSTEP = len(STEPS)


def build_nc():
    nc = bacc.Bacc("TRN2", target_bir_lowering=False, num_devices=NCORES)

    # xt: x for this core's 2 batches, feature-major [512, 2048]
    xt_d = nc.dram_tensor("xt", [INP, BPC * N], F16, kind="ExternalInput")
    # wqk: per head [q(64) | k(64)] -> [512, 8*128]
    wqk_d = nc.dram_tensor("wqk", [INP, H * 128], F16, kind="ExternalInput")
    # wv: per head v -> [512, 8*64]
    wv_d = nc.dram_tensor("wv", [INP, H * D], F16, kind="ExternalInput")
    # tab: per head 8 key-block slabs; row (h*1024 + mc*128 + p), col n.
    #   even mc rows hold exp(bias); odd mc rows hold A_EXP*bias
    tab_d = nc.dram_tensor("tab", [H * N, N], F16, kind="ExternalInput")
    wout_d = nc.dram_tensor("wout", [INNER, OUP], F16, kind="ExternalInput")
    bout_d = nc.dram_tensor("bout", [1, OUP], BF16, kind="ExternalInput")
    out_d = nc.dram_tensor("out", [BPC * N, OUP], F16, kind="ExternalOutput")

    xt_f = xt_d.reshape([INP * BPC * N])
    wqk_f = wqk_d.reshape([INP * H * 128])
    wv_f = wv_d.reshape([INP * H * D])
    tab_f = tab_d.reshape([H * N * N])
    wout_f = wout_d.reshape([INNER * OUP])

    with tile.TileContext(nc) as tc:
        with (
            tc.tile_pool(name="consts", bufs=1) as consts,
            tc.tile_pool(name="tab", bufs=2) as tabp,
            tc.tile_pool(name="qkt", bufs=2) as qktp,
            tc.tile_pool(name="ktq", bufs=2) as ktqp,
            tc.tile_pool(name="vaug", bufs=2) as vaugp,
            tc.tile_pool(name="attn", bufs=6) as attnp,
            tc.tile_pool(name="atti", bufs=6) as attip,
            tc.tile_pool(name="o2bt", bufs=2) as o2btp,
            tc.tile_pool(name="outp", bufs=3) as outp,
            tc.tile_pool(name="psdots", bufs=2, space="PSUM") as psdots,
            tc.tile_pool(name="pso", bufs=2, space="PSUM") as pso,
            tc.tile_pool(name="pswork", bufs=2, space="PSUM") as pswork,
        ):
            # ---- resident SBUF tensors ----
            xt_sb = consts.tile([128, 4, BPC * N], F16, tag="xt")
            wqk_sb = consts.tile([128, 4, H * 128], F16, tag="wqk")
            wv_sb = consts.tile([128, 4, H * D], F16, tag="wv")
            wout_sb = consts.tile([128, 4, OUP], F16, tag="wout")
            bout_sb = consts.tile([1, OUP], BF16, tag="bout")
            ones0 = consts.tile([1, 128], BF16, tag="ones0")
            # o2b_all[p, hpair, b, n]: rows 0-63 even head of the pair,
            # 64-127 odd head (odd halves arrive via sbuf-sbuf DMA).
            o2b_all = consts.tile([128, 4, BPC, N], F16, tag="o2b")
            rb_sb = consts.tile([64, 2, N], F32, tag="rb")   # 2-slot rotation
            dn_sb = consts.tile([1, 2, N], BF16, tag="dn")

            nc.vector.memset(ones0[:], 1.0)

            # ---- prologue DMAs ----
            nc.sync.dma_start(
                out=wqk_sb[:],
                in_=AP(wqk_f, 0, [[1024, 128], [128 * 1024, 4], [1, 1024]]))
            nc.sync.dma_start(
                out=xt_sb[:, :, 0:N],
                in_=AP(xt_f, 0, [[2048, 128], [128 * 2048, 4], [1, 1024]]))
            nc.gpsimd.dma_start(
                out=wv_sb[:],
                in_=AP(wv_f, 0, [[512, 128], [128 * 512, 4], [1, 512]]))
            nc.gpsimd.dma_start(
                out=xt_sb[:, :, N:2 * N],
                in_=AP(xt_f, 1024, [[2048, 128], [128 * 2048, 4], [1, 1024]]))
            nc.sync.dma_start(
                out=wout_sb[:],
                in_=AP(wout_f, 0, [[512, 128], [128 * 512, 4], [1, 512]]))
            nc.sync.dma_start(out=bout_sb[:], in_=bout_d[:])

            tab_tiles = {}

            def load_tab(h):
                t = tabp.tile([128, 8, N], F16, tag="tab")
                nc.scalar.dma_start(
                    out=t[:],
                    in_=AP(tab_f, h * N * N,
                           [[1024, 128], [128 * 1024, 8], [1, 1024]]))
                tab_tiles[h] = t

            load_tab(0)
            load_tab(1)

            # ---- per-step state ----
            qkt_of, ktq_of, vaug_of = {}, {}, {}
            o_ps_of = {}
            attn_store = {}
            pend = [None]   # (step, slot) awaiting normalization

            def project_qk(i):
                """q/k projection: qkt [q(0:64); k(64:128), n]."""
                h, b = STEPS[i]
                qkt = qktp.tile([128, N], F16)
                for fc in range(2):
                    ps = pswork.tile([128, 512], F32, tag="work")
                    for ic in range(4):
                        nc.tensor.matmul(
                            ps[:],
                            wqk_sb[:, ic, h * 128:(h + 1) * 128],
                            xt_sb[:, ic, b * N + fc * 512:b * N + (fc + 1) * 512],
                            start=(ic == 0), stop=(ic == 3))
                    nc.scalar.copy(qkt[:, fc * 512:(fc + 1) * 512], ps[:])
                qkt_of[i] = qkt

            def ktq_copies(i):
                """swapped copy for row-tiled dots: [k(0:64); q(64:128)]."""
                qkt = qkt_of[i]
                ktq = ktqp.tile([128, N], F16)
                nc.sync.dma_start(out=ktq[0:64, :], in_=qkt[64:128, :])
                nc.sync.dma_start(out=ktq[64:128, :], in_=qkt[0:64, :])
                ktq_of[i] = ktq

            def vaug_new(i):
                va = vaugp.tile([128, 8, D + 1], F16)
                nc.vector.memset(va[:, :, D], 1.0)
                vaug_of[i] = va

            def project_v(i, half):
                """v projection for 4 key-blocks (half 0: mc0-3, 1: mc4-7)."""
                h, b = STEPS[i]
                ps = pswork.tile([128, 4, D], F32, tag="work")
                for k in range(4):
                    mc = half * 4 + k
                    for ic in range(4):
                        nc.tensor.matmul(
                            ps[:, k, :],
                            xt_sb[:, ic, b * N + mc * 128:b * N + (mc + 1) * 128],
                            wv_sb[:, ic, h * D:(h + 1) * D],
                            start=(ic == 0), stop=(ic == 3))
                nc.scalar.copy(
                    vaug_of[i][:, half * 4:(half + 1) * 4, 0:D], ps[:])

            def dots(i, mc, fc):
                """dots half-block + its softmax ops."""
                h, _ = STEPS[i]
                qkt, ktq, tab = qkt_of[i], ktq_of[i], tab_tiles[h]
                ps = psdots.tile([128, 512], F32, tag="d")
                if mc % 2 == 0:
                    lhsT = ktq[0:64, mc * 128:(mc + 1) * 128]
                    rhs = qkt[0:64, fc * 512:(fc + 1) * 512]
                    tp = (0, 0)
                else:
                    lhsT = qkt[64:128, mc * 128:(mc + 1) * 128]
                    rhs = ktq[64:128, fc * 512:(fc + 1) * 512]
                    tp = (64, 0)
                nc.tensor.matmul(ps[:], lhsT, rhs,
                                 start=True, stop=True, tile_position=tp)
                if mc in A_MC:
                    ae = attnp.tile([128, 512], F16, tag="ae")
                    nc.scalar.activation(ae[:], ps[:],
                                         mybir.ActivationFunctionType.Exp,
                                         scale=SCALE)
                    at = attnp.tile([128, 512], F16, tag="at")
                    eng = nc.vector if mc == 2 else nc.gpsimd
                    eng.tensor_tensor(
                        at[:], ae[:], tab[:, mc, fc * 512:(fc + 1) * 512],
                        mybir.AluOpType.mult)
                    attn_store[(i, mc, fc)] = at[:]
                else:
                    ai = attip.tile([128, 512], I16, tag="ai")
                    nc.vector.affine_then_add(
                        out=ai[:], in0=ps[:],
                        in1=tab[:, mc, fc * 512:(fc + 1) * 512],
                        scale=SCALE * A_EXP, bias=C1_EXP)
                    attn_store[(i, mc, fc)] = ai[:].bitcast(F16)

            def pv(i, mc, fc, stop=False):
                if i not in o_ps_of:
                    o_ps_of[i] = pso.tile([D + 1, N], F32, tag="o",
                                          name=f"o_ps_{i}")
                a = attn_store.pop((i, mc, fc))
                nc.tensor.matmul(
                    o_ps_of[i][:, fc * 512:(fc + 1) * 512],
                    vaug_of[i][:, mc, :], a,
                    start=(mc == 0), stop=stop)

            def norm_mm(pstate):
                """PE part of the deferred normalization."""
                _, slot = pstate
                for fc in range(2):
                    rbps = pswork.tile([64, 512], F32, tag="work")
                    nc.tensor.matmul(
                        rbps[:],
                        ones0[0:1, 0:64],
                        dn_sb[0:1, slot, fc * 512:(fc + 1) * 512],
                        start=True, stop=True)
                    nc.vector.reciprocal_approx_fast(
                        rb_sb[:, slot, fc * 512:(fc + 1) * 512], rbps[:])

            def norm_rest(pstate):
                """DVE multiply (+ odd-head partition-shift DMA)."""
                i, slot = pstate
                h, b = STEPS[i]
                o_ps = o_ps_of.pop(i)
                pair = h // 2
                if h % 2 == 0:
                    nc.vector.tensor_tensor(
                        o2b_all[0:64, pair, b, :], o_ps[0:64, :],
                        rb_sb[:, slot, :], mybir.AluOpType.mult)
                else:
                    o2t = o2btp.tile([64, N], F16, tag="o2t")
                    nc.vector.tensor_tensor(
                        o2t[:], o_ps[0:64, :],
                        rb_sb[:, slot, :], mybir.AluOpType.mult)
                    nc.sync.dma_start(out=o2b_all[64:128, pair, b, :],
                                      in_=o2t[:])

            def out_proj(b, chunks):
                """output projection for batch b over the given pos-chunks."""
                for pc in chunks:
                    ps = pswork.tile([128, 512], F32, tag="work")
                    nc.tensor.matmul(ps[:], ones0[0:1, :], bout_sb[:],
                                     start=True, stop=False)
                    for kc in range(4):
                        nc.tensor.matmul(
                            ps[:],
                            o2b_all[:, kc, b, pc * 128:(pc + 1) * 128],
                            wout_sb[:, kc, :],
                            start=False, stop=(kc == 3))
                    o_sb = outp.tile([128, OUP], F16, tag="osb")
                    nc.scalar.copy(o_sb[:], ps[:])
                    nc.sync.dma_start(
                        out=out_d[b * N + pc * 128:b * N + (pc + 1) * 128, :],
                        in_=o_sb[:])

            # ---- prologue: projections + first dots of step 0 ----
            project_qk(0)
            ktq_copies(0)
            vaug_new(0)
            project_v(0, 0)
            project_v(0, 1)
            dots(0, 0, 0); dots(0, 1, 0)
            dots(0, 0, 1); dots(0, 1, 1)

            # ---- main loop ----
            for i in range(NSTEP):
                h, b = STEPS[i]
                last = i == NSTEP - 1
                # prefetch bias table two steps ahead
                if i + 2 < NSTEP:
                    h2 = STEPS[i + 2][0]
                    if h2 not in tab_tiles:
                        for hh in list(tab_tiles):
                            if hh < h:
                                del tab_tiles[hh]
                        load_tab(h2)

                dots(i, 2, 0); dots(i, 3, 0)
                pv(i, 0, 0); pv(i, 1, 0)
                if not last:
                    project_qk(i + 1)               # 8 mm + 2 copies
                dots(i, 2, 1); dots(i, 3, 1)
                pv(i, 0, 1); pv(i, 1, 1)
                if not last:
                    vaug_new(i + 1)
                    project_v(i + 1, 0)             # 16 small mm
                dots(i, 4, 0); dots(i, 5, 0)
                pv(i, 2, 0); pv(i, 3, 0)
                if pend[0] is not None:
                    norm_mm(pend[0])                # 2 tiny mm + recips
                dots(i, 4, 1); dots(i, 5, 1)
                pv(i, 2, 1); pv(i, 3, 1)
                if not last:
                    project_v(i + 1, 1)             # 16 small mm
                dots(i, 6, 0); dots(i, 7, 0)
                pv(i, 4, 0); pv(i, 5, 0)
                if pend[0] is not None:
                    norm_rest(pend[0])              # DVE mult + odd-h DMA
                    pend[0] = None
                if last:
                    out_proj(0, range(0, 4))
                dots(i, 6, 1); dots(i, 7, 1)
                pv(i, 4, 1); pv(i, 5, 1)
                if not last:
                    ktq_copies(i + 1)
                else:
                    out_proj(0, range(4, 8))
                # next step's first two key-blocks ride this step's tail
                if not last:
                    dots(i + 1, 0, 0); dots(i + 1, 1, 0)
                pv(i, 6, 0); pv(i, 7, 0)
                if not last:
                    dots(i + 1, 0, 1); dots(i + 1, 1, 1)
                pv(i, 6, 1); pv(i, 7, 1, stop=True)

                # denominator row -> bf16 for the deferred normalize
                slot = i % 2
                nc.scalar.copy(dn_sb[0:1, slot, :],
                               o_ps_of[i][D:D + 1, :])
                pend[0] = (i, slot)

            # ---- tail: last normalize + batch 1 projection ----
            norm_mm(pend[0])
            norm_rest(pend[0])
            out_proj(1, range(0, 8))

    nc.finalize()
    return nc


_NC_CACHE = None


def _get_nc():
    global _NC_CACHE
    if _NC_CACHE is None:
        _NC_CACHE = build_nc()
    return _NC_CACHE


def make_in_maps(x, W_qkv, rel_table, W_out, b_out):
    x = np.asarray(x, np.float32)
    W_qkv = np.asarray(W_qkv, np.float32)
    W_out = np.ascontiguousarray(np.asarray(W_out, np.float32)).astype(np.float16)
    b_out = np.ascontiguousarray(
        np.asarray(b_out, np.float32).reshape(1, OUP)).astype(ml_dtypes.bfloat16)
    rel_table = np.asarray(rel_table, np.float32)

    # per-head reorganized projection weights
    wqk = np.empty((INP, H * 128), np.float16)
    wv = np.empty((INP, H * D), np.float16)
    for h in range(H):
        wqk[:, h * 128:h * 128 + 64] = W_qkv[:, h * D:(h + 1) * D]
        wqk[:, h * 128 + 64:h * 128 + 128] = \
            W_qkv[:, INNER + h * D:INNER + (h + 1) * D]
        wv[:, h * D:(h + 1) * D] = \
            W_qkv[:, 2 * INNER + h * D:2 * INNER + (h + 1) * D]

    # bias^T[m, n] = rel_table[idx(n, m)]: full per-(m, n) table per head
    mprime = 63 * (np.arange(N) // 32) + (np.arange(N) % 32)
    idx = 1984 - mprime[:, None] + mprime[None, :]  # [m, n]
    tab = np.empty((H * N, N), np.float16)
    for h in range(H):
        tcol = np.zeros(1984 + 2048, np.float32)
        tcol[:TBL] = rel_table[:, h]
        bias_full = tcol[idx]                       # [m, n] f32
        for mc in range(8):
            rows = bias_full[mc * 128:(mc + 1) * 128]
            if mc in A_MC:
                tab[h * N + mc * 128:h * N + (mc + 1) * 128] = np.exp(rows)
            else:
                tab[h * N + mc * 128:h * N + (mc + 1) * 128] = A_EXP * rows

    in_maps = []
    for c in range(NCORES):
        xt2 = np.ascontiguousarray(
            x[BPC * c:BPC * (c + 1)].reshape(BPC * N, INP).T).astype(np.float16)
        in_maps.append({
            "xt": xt2, "wqk": wqk, "wv": wv, "tab": tab,
            "wout": W_out, "bout": b_out,
        })
    return in_maps


def run(inputs, trace=False, **kw):
    nc = _get_nc()
    in_maps = make_in_maps(inputs["x"], inputs["W_qkv"], inputs["rel_table"],
                           inputs["W_out"], inputs["b_out"])
    res = run_bass_kernel_spmd(nc, in_maps, core_ids=list(range(NCORES)),
                               trace=trace, **kw)
    out = np.empty((B, N, OUP), np.float32)
    for c in range(NCORES):
        out[BPC * c:BPC * (c + 1)] = \
            res.results[c]["out"].astype(np.float32).reshape(BPC, N, OUP)
    return out, res


def kernel(**inputs):
    out, _ = run(inputs, trace=False)
    return out


# revision 10
# speedup vs baseline: 1.2041x; 1.2041x over previous
"""Trainium2 Bass kernel for windowless relative-position-bias attention.

Problem (hardcoded shapes):
  x [16, 1024, 512] f32, W_qkv [512, 1536], rel_table [3969, 8],
  W_out [512, 512], b_out [512], rel_index [1048576] i32 (canonical
  32x32 relative-position pattern; only its structure is used).

Sharding: tensor-parallel over heads -- core c owns head c for all 16
batches; the final projection is data-parallel over batches (core c
produces output batches 2c, 2c+1) after an on-chip AllToAll of the
per-head attention outputs.

Per core (head h = core id) the 16 batches run as a 3-deep software
pipeline: iteration i emits batch b_i's softmax+PV, batch b_{i+1}'s
projections (qT/kT/v) interleaved into the PE stream, and batch
b_{i-1}'s normalization -- so TensorE never idles at batch boundaries
and stays at the warm 2.4 GHz HAM clock.

Softmax (max-subtraction skipped; logits bounded ~|9|) splits the
exp+bias work per mc-block across three engines:
  * a-blocks: ScalarE exp(SCALE*dots), then multiply with the
    host-precomputed exp(bias) on VectorE or GpSimd.
  * d-blocks: fused Schraudolph exp on VectorE -- one custom DVE op
    i16 = (dots*SCALE*A + C1) + A*bias, bits reinterpreted as fp16
    (A = 1024*log2(e); C1 centers the piecewise-linear exp2 error so
    softmax ratios stay unbiased; adds ~0.8% rel err, budget is 2%).
The PV matmul gets the softmax denominator for free via a ones-column
appended to v; normalization is a K=1 reciprocal-broadcast matmul.

AllToAll halves: even batches exchange while odd batches compute; the
half-A output projection is interleaved into late iterations, only
half-B remains in the tail.  A tiny warm-up AllToAll doubles as a
cross-core start-skew barrier: batch 0's PV consumes one value from it.
"""

import os
import sys

for _p in ("/opt/trn_rl_repo", "/root/.axon_site/_ro/trn_rl_repo"):
    if os.path.isdir(_p) and _p not in sys.path:
        sys.path.insert(0, _p)

import numpy as np
import ml_dtypes

import concourse.bass as bass
import concourse.mybir as mybir
import concourse.tile as tile
from concourse import bacc
from concourse.bass import AP
from concourse.bass_utils import run_bass_kernel_spmd

# Content-hash NEFF cache: identical BIR -> reuse the compiled NEFF
# (neuronxcc is ~6 min; this makes repeat runs seconds).
import concourse.bass_utils as _bu
import concourse.bass2jax as _b2j

_orig_compile_bir = _bu.compile_bir_kernel


def _cached_compile_bir(bir_json, tmpdir, neff_name="file.neff"):
    import hashlib
    import shutil
    h = hashlib.sha256(bir_json).hexdigest()[:24]
    cdir = os.environ.get("NEFF_CACHE_DIR", "/tmp/neff_cache")
    os.makedirs(cdir, exist_ok=True)
    cpath = os.path.join(cdir, h + ".neff")
    if os.path.exists(cpath):
        dst = os.path.join(tmpdir, neff_name)
        shutil.copy(cpath, dst)
        return dst
    p = _orig_compile_bir(bir_json, tmpdir, neff_name)
    try:
        shutil.copy(p, cpath)
    except OSError:
        pass
    return p


_bu.compile_bir_kernel = _cached_compile_bir
_b2j.compile_bir_kernel = _cached_compile_bir

B, IH, IW = 16, 32, 32
N = IH * IW          # 1024
H, D = 8, 64
INNER = H * D        # 512
INP = OUP = 512
SCALE = D ** -0.5    # 0.125
NCORES = 8
BPC = B // NCORES    # batches per core = 2
TBL = (2 * IH - 1) * (2 * IW - 1)  # 3969

# Schraudolph fp16 exp constants
A_EXP = 1024.0 * np.log2(np.e)          # 1477.3197
C1_EXP = 15360.0 - 59.0 + 0.5           # exponent offset - mean-center - trunc

# per-batch mc-block engine assignment (tunable):
A_DVE = (0,)          # ScalarE exp + VectorE bias-mult
A_GPS = (1, 2)        # ScalarE exp + GpSimd bias-mult
D_SCH = (3, 4, 5, 6, 7)   # VectorE fused Schraudolph (incl. bias)
A_BLK = A_DVE + A_GPS

F32 = mybir.dt.float32
F32R = mybir.dt.float32r
F16 = mybir.dt.float16
BF16 = mybir.dt.bfloat16
I16 = mybir.dt.int16


def build_nc():
    nc = bacc.Bacc("TRN2", target_bir_lowering=False, num_devices=NCORES)

    xt_d = nc.dram_tensor("xt", [INP, B * N], F16, kind="ExternalInput")
    wqk_d = nc.dram_tensor("wqk", [INP, 128], F16, kind="ExternalInput")
    wv_d = nc.dram_tensor("wv", [INP, D], F16, kind="ExternalInput")
    texp_d = nc.dram_tensor("texp", [len(A_BLK) * 128, N], F16, kind="ExternalInput")
    tba_d = nc.dram_tensor("tba", [len(D_SCH) * 128, N], F16, kind="ExternalInput")
    wout_d = nc.dram_tensor("wout", [INNER, OUP], F16, kind="ExternalInput")
    bout_d = nc.dram_tensor("bout", [1, OUP], BF16, kind="ExternalInput")
    ones_d = nc.dram_tensor("ones", [1024], BF16, kind="ExternalInput")
    out_d = nc.dram_tensor("out", [BPC * N, OUP], F32, kind="ExternalOutput")

    with tile.TileContext(nc) as tc:
        with (
            tc.tile_pool(name="consts", bufs=1) as consts,
            tc.tile_pool(name="xt", bufs=3) as xtp,
            tc.tile_pool(name="qkt", bufs=2) as qktp,
            tc.tile_pool(name="ktq", bufs=2) as ktqp,
            tc.tile_pool(name="vaug", bufs=2) as vaugp,
            tc.tile_pool(name="attn", bufs=4) as attnp,
            tc.tile_pool(name="atti", bufs=3) as attip,
            tc.tile_pool(name="small", bufs=2) as smallp,
            tc.tile_pool(name="o2b", bufs=2) as o2bp,
            tc.tile_pool(name="lh", bufs=1) as lhp,
            tc.tile_pool(name="outp", bufs=2) as outp,
            tc.tile_pool(name="pswork", bufs=3, space="PSUM") as pswork,
            tc.tile_pool(name="psacc", bufs=1, space="PSUM") as psacc,
            tc.tile_pool(name="dram", bufs=1, space="DRAM") as dramp,
        ):
            batch_order = list(range(0, B, 2)) + list(range(1, B, 2))

            # ---- x^T for batches 0,1 queued before anything slow ----
            def load_xt(b):
                xt = xtp.tile([128, 4, N], F16)
                for ic in range(4):
                    eng = nc.sync if ic < 2 else nc.gpsimd
                    eng.dma_start(
                        out=xt[:, ic, :],
                        in_=xt_d[ic * 128:(ic + 1) * 128, b * N:(b + 1) * N])
                return xt

            xt_tiles = {0: load_xt(batch_order[0]),
                        1: load_xt(batch_order[1])}

            # ---- collectives warm-up; also the cross-core skew barrier
            # (batch 0's PV consumes one exchanged value) ----
            cc_inA = dramp.tile([NCORES, D, N], F16, tag="ccinA")
            cc_outA = dramp.tile([NCORES, D, N], F16, tag="ccoutA")
            cc_inB = dramp.tile([NCORES, D, N], F16, tag="ccinB")
            cc_outB = dramp.tile([NCORES, D, N], F16, tag="ccoutB")
# no warm-up collective: its peer-skew wait serializes DMA-ring traffic
            # behind it for ~25us at startup; A2A-A at i=8 absorbs firmware
            # init asynchronously (3 iterations of slack before lhA is read)

            # ---- weights ----
            wqk_sb = consts.tile([128, 4, 128], F16, tag="wqk")
            wv_sb = consts.tile([128, 4, D], F16, tag="wv")
            wout_sb = consts.tile([128, 4, OUP], F16, tag="wout")
            bout_sb = consts.tile([65, OUP], BF16, tag="bout")
            ones1 = consts.tile([65, 128], BF16, tag="ones")
            ones0 = consts.tile([1, 128], BF16, tag="ones0")
            for ic in range(4):
                nc.sync.dma_start(out=wqk_sb[:, ic, :], in_=wqk_d[ic * 128:(ic + 1) * 128, :])
                nc.sync.dma_start(out=wv_sb[:, ic, :], in_=wv_d[ic * 128:(ic + 1) * 128, :])
                nc.gpsimd.dma_start(out=wout_sb[:, ic, :], in_=wout_d[ic * 128:(ic + 1) * 128, :])
            nc.sync.dma_start(out=bout_sb[64:65, :], in_=bout_d[:])
            nc.sync.dma_start(out=ones1[64:65, :], in_=ones_d[0:128])
            nc.sync.dma_start(out=ones0[:], in_=ones_d[0:128])

            # ---- bias tables (host-expanded per-(m,n); contiguous DMA) ----
            expb = consts.tile([128, len(A_BLK), N], F16, tag="expb")
            biasA = consts.tile([128, len(D_SCH), N], F16, tag="biasA")
            for slot in range(len(A_BLK)):
                nc.sync.dma_start(out=expb[:, slot, :],
                                  in_=texp_d[slot * 128:(slot + 1) * 128, :])
            for slot in range(len(D_SCH)):
                nc.gpsimd.dma_start(out=biasA[:, slot, :],
                                    in_=tba_d[slot * 128:(slot + 1) * 128, :])

            lhA = lhp.tile([128, 4, N], F16, tag="lhA")
            lhB = lhp.tile([128, 4, N], F16, tag="lhB")

            def out_chunk(nq, lh):
                """output-projection chunk: out rows nq*128..+128."""
                ps_f = pswork.tile([128, OUP], F32, tag="big")
                for kc in range(4):
                    nc.tensor.matmul(
                        ps_f[:],
                        lh[:, kc, (nq % 8) * 128:(nq % 8 + 1) * 128],
                        wout_sb[:, kc, :],
                        start=(kc == 0), stop=False)
                nc.tensor.matmul(ps_f[:], ones1[64:65, :], bout_sb[64:65, :],
                                 start=False, stop=True)
                o_sb = outp.tile([128, OUP], F32)
                nc.scalar.copy(o_sb[:], ps_f[:])
                nc.sync.dma_start(out=out_d[nq * 128:(nq + 1) * 128, :],
                                  in_=o_sb[:])

            def normalize(state):
                """1/denominator broadcast + multiply + ship to cc buffer.

                The K=1 broadcast matmul runs in bf16 (an f32/f32r one
                lowers to the 4-cycle/row two-pass fp32 path, ~1.4us of
                TensorE per batch; bf16 is full rate and the denominator
                only needs ~3 digits).
                """
                o_ps, dn, b = state
                rb_ps = pswork.tile([D, N], F32, tag="big")
                for fc in range(2):
                    nc.tensor.matmul(
                        rb_ps[:, fc * 512:(fc + 1) * 512],
                        ones0[0:1, 0:D],
                        dn[0:1, fc * 512:(fc + 1) * 512],
                        start=True, stop=True)
                rb = smallp.tile([D, N], F32, tag="rb")
                nc.vector.reciprocal_approx_fast(rb[:], rb_ps[:])
                o2b = o2bp.tile([D, N], F16)
                nc.vector.tensor_tensor(o2b[:], o_ps[0:D, :], rb[:],
                                        mybir.AluOpType.mult)
                cc_dst = cc_inA if b % 2 == 0 else cc_inB
                nc.sync.dma_start(out=cc_dst[b // BPC], in_=o2b[:])

            def project(b, xt, barrier=False):
                """qT/kT + v for batch b; returns (qkt, ktq_lo_pending, vaug).

                Emits the PE matmuls + casts + the sync-queue k copy; the
                gpsimd-queue q duplicate is deferred (emit_ktq_hi) so it
                lands after this iteration's gpsimd multiplies.
                """
                qkt_ps = pswork.tile([128, N], F32, tag="big")
                for fc in range(2):
                    for ic in range(4):
                        nc.tensor.matmul(
                            qkt_ps[:, fc * 512:(fc + 1) * 512],
                            wqk_sb[:, ic, :],
                            xt[:, ic, fc * 512:(fc + 1) * 512],
                            start=(ic == 0), stop=(ic == 3))
                qkt = qktp.tile([128, N], F16)
                nc.scalar.copy(qkt[:], qkt_ps[:])
                ktq = ktqp.tile([128, N], F16)
                nc.sync.dma_start(out=ktq[0:64, :], in_=qkt[64:128, :])
                return qkt, ktq

            def project_v(b, xt):
                vaug = vaugp.tile([128, 8, D + 1], F16)
                nc.vector.memset(vaug[:, :, D], 1.0)
                v_ps = pswork.tile([128, 8, D], F32, tag="big")
                for nc_ in range(8):
                    for ic in range(4):
                        nc.tensor.matmul(
                            v_ps[:, nc_, :],
                            xt[:, ic, nc_ * 128:(nc_ + 1) * 128],
                            wv_sb[:, ic, :],
                            start=(ic == 0), stop=(ic == 3))
                nc.scalar.copy(vaug[:, :, 0:D], v_ps[:])
                return vaug

            # ---- prologue: batch 0 projections ----
            qkt_cur, ktq_cur = project(batch_order[0], xt_tiles[0])
            nc.gpsimd.dma_start(out=ktq_cur[64:128, :], in_=qkt_cur[0:64, :])
            vaug_cur = project_v(batch_order[0], xt_tiles[0])

            pend = None  # deferred normalize state

            for i, b in enumerate(batch_order):
                xt = xt_tiles.pop(i)
                last = i == B - 1
                nxt = batch_order[i + 1] if not last else None
                qkt, ktq, vaug = qkt_cur, ktq_cur, vaug_cur

                # --- normalization of the previous batch (PE: 2 tiny MMs) ---
                if pend is not None:
                    normalize(pend)
                    pend = None
                    if i == B // 2:  # last even batch just shipped
                        nc.gpsimd.collective_compute(
                            "AllToAll", mybir.AluOpType.bypass,
                            replica_groups=[list(range(NCORES))],
                            ins=[cc_inA.opt()], outs=[cc_outA.opt()])
                if i == 11:
                    for kc in range(4):
                        src = AP(cc_outA.rearrange("h d n -> (h d n)").tensor,
                                 kc * 128 * N, [[N, 128], [1, N]])
                        nc.sync.dma_start(out=lhA[:, kc, :], in_=src)

                o_ps = psacc.tile([D + 1, N], F32, tag="o")
                attn_of = {}

                def dots(mc):
                    ps = pswork.tile([128, N], F32, tag="big")
                    if mc % 2 == 0:
                        lhsT, rhs, tp = ktq[0:64, mc * 128:(mc + 1) * 128], \
                            qkt[0:64, :], (0, 0)
                    else:
                        lhsT, rhs, tp = qkt[64:128, mc * 128:(mc + 1) * 128], \
                            ktq[64:128, :], (64, 0)
                    for fc in range(2):
                        nc.tensor.matmul(
                            ps[:, fc * 512:(fc + 1) * 512],
                            lhsT, rhs[:, fc * 512:(fc + 1) * 512],
                            start=True, stop=True, tile_position=tp)
                    return ps

                def softmax_block(mc, ps):
                    if mc in A_BLK:
                        attn_e = attnp.tile([128, N], F16, tag="attn_e")
                        nc.scalar.activation(attn_e[:], ps[:],
                                             mybir.ActivationFunctionType.Exp,
                                             scale=SCALE)
                        attn = attnp.tile([128, N], F16, tag="attn")
                        slot = A_BLK.index(mc)
                        eng = nc.vector if mc in A_DVE else nc.gpsimd
                        eng.tensor_tensor(attn[:], attn_e[:], expb[:, slot, :],
                                          mybir.AluOpType.mult)
                        attn_of[mc] = attn[:]
                    else:
                        slot = D_SCH.index(mc)
                        atti = attip.tile([128, N], I16)
                        nc.vector.affine_then_add(
                            out=atti[:], in0=ps[:], in1=biasA[:, slot, :],
                            scale=SCALE * A_EXP, bias=C1_EXP)
                        attn_of[mc] = atti[:].bitcast(F16)

                first_pv = [True]

                def pv(mc, stop=False):
                    a = attn_of.pop(mc)
                    st = first_pv[0]
                    for fc in range(2):
                        nc.tensor.matmul(
                            o_ps[:, fc * 512:(fc + 1) * 512],
                            vaug[:, mc, :],
                            a[:, fc * 512:(fc + 1) * 512],
                            start=st, stop=stop)
                    first_pv[0] = False

                def d_and_sm(mc):
                    softmax_block(mc, dots(mc))

                # --- interleaved PE stream for this iteration ---
                d_and_sm(0)
                d_and_sm(1)
                if nxt is not None:
                    qkt_cur, ktq_cur = project(nxt, xt_tiles[i + 1])
                d_and_sm(2)
                d_and_sm(3)
                pv(0)
                pv(3)
                if nxt is not None:
                    vaug_cur = project_v(nxt, xt_tiles[i + 1])
                d_and_sm(4)
                pv(1)
                d_and_sm(5)
                pv(4)
                d_and_sm(6)
                pv(2)
                d_and_sm(7)
                pv(5)
                pv(6)
                pv(7, stop=True)

                # denominator row for the deferred normalize
                dn = smallp.tile([1, N], BF16, tag="dn")
                nc.scalar.copy(dn[:], o_ps[D:D + 1, :])
                pend = (o_ps, dn, b)

                # late gpsimd-queue work: q duplicate + x^T prefetch
                if nxt is not None:
                    nc.gpsimd.dma_start(out=ktq_cur[64:128, :],
                                        in_=qkt_cur[0:64, :])
                if i + 2 < B:
                    xt_tiles[i + 2] = load_xt(batch_order[i + 2])

                # half-A output projection rides late odd iterations
                if i >= 12:
                    for nq in range(2 * (i - 12), 2 * (i - 11)):
                        out_chunk(nq, lhA)

            # ---- tail: last normalize, half-B exchange + projection ----
            normalize(pend)
            nc.gpsimd.collective_compute(
                "AllToAll", mybir.AluOpType.bypass,
                replica_groups=[list(range(NCORES))],
                ins=[cc_inB.opt()], outs=[cc_outB.opt()])
            for kc in range(4):
                src = AP(cc_outB.rearrange("h d n -> (h d n)").tensor,
                         kc * 128 * N, [[N, 128], [1, N]])
                nc.gpsimd.dma_start(out=lhB[:, kc, :], in_=src)
            for nq in range(8, 16):
                out_chunk(nq, lhB)

    nc.finalize()
    return nc


_NC_CACHE = None


def _get_nc():
    global _NC_CACHE
    if _NC_CACHE is None:
        _NC_CACHE = build_nc()
    return _NC_CACHE


def make_in_maps(x, W_qkv, rel_table, W_out, b_out):
    xt2 = np.ascontiguousarray(
        np.asarray(x, np.float32).reshape(B * N, INP).T).astype(np.float16)
    W_qkv = np.asarray(W_qkv, np.float32)
    W_out = np.ascontiguousarray(np.asarray(W_out, np.float32)).astype(np.float16)
    b_out = np.ascontiguousarray(
        np.asarray(b_out, np.float32).reshape(1, OUP)).astype(ml_dtypes.bfloat16)
    rel_table = np.asarray(rel_table, np.float32)
    # bias^T[m, n] = rel_table[idx(n, m)]: full per-(m, n) index table
    mprime = 63 * (np.arange(N) // 32) + (np.arange(N) % 32)
    idx = 1984 - mprime[:, None] + mprime[None, :]  # [m, n]
    in_maps = []
    for c in range(NCORES):
        wqk = np.ascontiguousarray(np.concatenate(
            [W_qkv[:, c * D:(c + 1) * D],
             W_qkv[:, INNER + c * D:INNER + (c + 1) * D]], axis=1)).astype(np.float16)
        wv = np.ascontiguousarray(
            W_qkv[:, 2 * INNER + c * D:2 * INNER + (c + 1) * D]
        ).astype(np.float16)
        tcol = np.zeros(1984 + 2048, np.float32)
        tcol[:TBL] = rel_table[:, c]
        bias_full = tcol[idx]                       # [m, n] f32
        texp = np.empty((len(A_BLK) * 128, N), np.float16)
        for slot, mc in enumerate(A_BLK):
            texp[slot * 128:(slot + 1) * 128] = \
                np.exp(bias_full[mc * 128:(mc + 1) * 128])
        tba = np.empty((len(D_SCH) * 128, N), np.float16)
        for slot, mc in enumerate(D_SCH):
            tba[slot * 128:(slot + 1) * 128] = \
                A_EXP * bias_full[mc * 128:(mc + 1) * 128]
        in_maps.append({
            "xt": xt2, "wqk": wqk, "wv": wv,
            "texp": texp, "tba": tba,
            "wout": W_out, "bout": b_out,
            "ones": np.ones(1024, ml_dtypes.bfloat16),
        })
    return in_maps


def run(inputs, trace=False, **kw):
    nc = _get_nc()
    in_maps = make_in_maps(inputs["x"], inputs["W_qkv"], inputs["rel_table"],
                           inputs["W_out"], inputs["b_out"])
    res = run_bass_kernel_spmd(nc, in_maps, core_ids=list(range(NCORES)),
                               trace=trace, **kw)
    out = np.empty((B, N, OUP), np.float32)
    for c in range(NCORES):
        out[BPC * c:BPC * (c + 1)] = res.results[c]["out"].reshape(BPC, N, OUP)
    return out, res


def kernel(**inputs):
    out, _ = run(inputs, trace=False)
    return out

